# revision 20
# baseline (speedup 1.0000x reference)
"""Trainium2 Bass kernel for nn_Attention_55362128445856 (v3).

Dense multi-head attention (B=8, N=1024, C=768, H=12, d=64) with LoRA on the
QKV projection and on the output-projection *output*.

Sharding: pure data-parallel over batch - core b computes batch element b.
Weights are replicated to every core.

v3 design (on top of v2):
  - LoRA folded into the weights ON HOST (exact):
      W_qkv_eff = W_qkv + 8*A_qkv@B_qkv
      W_proj_eff = W_proj @ (I + 8*A_proj@B_proj)
      b_eff = b_proj + 8*B_proj^T (A_proj^T b_proj)
  - ALL matmul operands in bf16 (PSUM accumulate stays fp32): halves DMA
    bytes, enables FWL fast weight loads, 2x DVE modes on copies/muls.
  - scores PSUM split per head: sc_A/sc_B [128,1024] (2 banks each) so the
    exp of head A overlaps the score matmuls of head B and the next jt's
    scores only WAR-wait on their own head's exp (v2 had one [128,2048]
    buffer serializing scores->exp->scores).
  - exp issued per head immediately after that head's two score MMs.
  - softmax denominators via ones-column in v ("v_aug", M=65).
  - phase-3 bias add on DVE (tensor_scalar per-partition) instead of ACT.
  - output leaves device as bf16 [C,N]; host converts/transposes.
"""

import numpy as np
from contextlib import ExitStack

import jax
import concourse.bass as bass
import concourse.bacc as bacc
import concourse.mybir as mybir
import concourse.tile as tile

B, N, C = 8, 1024, 768
H, D = 12, 64
R = 4
LORA_SCALING = 8.0
P = 128
CT = C // P          # 6 contraction tiles over C
TT = N // P          # 8 token tiles of 128
T2 = N // 512        # 2 token tiles of 512
F32 = mybir.dt.float32
F32R = mybir.dt.float32r
BF16 = mybir.dt.bfloat16
N_CORES = 8


def _pin_act_table():
    """Force every activation onto the one table set that holds Exp, Ln,
    Identity and Copy together, so the Ln/Exp reciprocal never thrashes
    table loads against the softmax Exp (1.28us per reload)."""
    import concourse.bacc as bacc_mod
    import concourse.hw_specs as hw_specs_mod
    if getattr(bacc_mod, "_act_tables_pinned", False):
        return
    orig = hw_specs_mod.get_activation_tables

    def pinned(arch):
        t = orig(arch)
        return {name: (s if name == "natural_log_exp_and_others" else set())
                for name, s in t.items()}

    bacc_mod.get_activation_tables = pinned
    bacc_mod._act_tables_pinned = True


def build_nc(debug=False, repeat=1, phases=(1, 2, 3), ph2_parts="full",
             exp_split=False):
    _pin_act_table()
    nc = bacc.Bacc("TRN2", target_bir_lowering=False, debug=debug,
                   num_devices=N_CORES)

    # x arrives pre-transposed from host: [C, N] feature-major, bf16
    x_d = nc.dram_tensor("x", [C, N], BF16, kind="ExternalInput").ap()
    wqkv_d = nc.dram_tensor("W_qkv", [C, 3 * C], BF16, kind="ExternalInput").ap()
    wproj_d = nc.dram_tensor("W_proj", [C, C], BF16, kind="ExternalInput").ap()
    bproj_d = nc.dram_tensor("b_proj", [C], F32, kind="ExternalInput").ap()
    # output leaves feature-major: [C, N] bf16
    out_d = nc.dram_tensor("out", [C, N], BF16, kind="ExternalOutput").ap()

    with tile.TileContext(nc) as tc, ExitStack() as ctx:
        const = ctx.enter_context(tc.tile_pool(name="const", bufs=1))

        # gpsimd custom-op library for partition_broadcast (normalization)
        from concourse import library_config
        nc.gpsimd.load_library(library_config.attn)

        ones_f = const.tile([P, H], BF16, tag="ones_f")
        nc.vector.memset(ones_f[:], 1.0)

        # b_eff as [128, 6]: column a holds b_eff[a*128 : (a+1)*128]
        bvec = const.tile([P, CT], F32, tag="bvec")
        nc.sync.dma_start(bvec[:], bproj_d.rearrange("(a p) -> p a", p=P))

        # persistent tiles
        xpool = ctx.enter_context(tc.tile_pool(name="xpool", bufs=1))
        xT = [xpool.tile([P, N], BF16, tag=f"xT{ct}", name=f"xT{ct}")
              for ct in range(CT)]
        wppool = ctx.enter_context(tc.tile_pool(name="wppool", bufs=1))
        wp_tiles = [wppool.tile([P, C], BF16, tag=f"wp{ct}", name=f"wp{ct}")
                    for ct in range(CT)]
        qkpool = ctx.enter_context(tc.tile_pool(name="qkpool", bufs=1))
        # qkT[0..5] = q feature-major (head pairs), qkT[6..11] = k
        qkT = [qkpool.tile([P, N], BF16, tag=f"qkT{i}", name=f"qkT{i}")
               for i in range(12)]
        # v token-major, interleaved per head with a ones column:
        # v_all[tt][:, h*65 : h*65+64] = v[tt*128:+128, head h], col h*65+64 = 1
        vpool = ctx.enter_context(tc.tile_pool(name="vpool", bufs=1))
        v_all = [vpool.tile([P, H * (D + 1)], BF16, tag=f"v{t}", name=f"v{t}")
                 for t in range(TT)]
        # attnT aliases the q tiles: qkT[hp] is dead once pair hp's scores
        # are done, exactly when attnT[hp] gets written
        attnT = qkT[:CT]

        # phase-gated builds: init tiles whose producer phase is skipped
        if 1 not in phases or 2 not in phases:
            for i in range(12):
                nc.vector.memset(qkT[i].bitcast(F32)[:], 0.0)
        if 1 not in phases:
            for t in range(TT):
                nc.vector.memset(v_all[t].bitcast(F32)[:], 0.0)
            if 3 in phases:
                for ct in range(CT):
                    nc.vector.memset(wp_tiles[ct].bitcast(F32)[:], 0.0)

        # ---------------- Phase 1: QKV projection ----------------
        with tc.tile_pool(name="ph1", bufs=1) as ph1, \
             tc.tile_pool(name="pp_qk", bufs=3, space="PSUM") as pp_qk, \
             tc.tile_pool(name="pp_v", bufs=2, space="PSUM") as pp_v:
            for _rep in range(repeat if 1 in phases else 0):

                w_tiles = [ph1.tile([P, 3 * C], BF16, tag=f"w{ct}",
                                    name=f"w{ct}") for ct in range(CT)]

                # DMA order == consumption order, all on the SP HWDGE queue:
                # (xT[ct], Wv[ct]) interleaved, then q/k columns pair-major,
                # then W_proj
                for ct in range(CT):
                    nc.sync.dma_start(xT[ct][:], x_d[ct * P:(ct + 1) * P, :])
                    nc.sync.dma_start(w_tiles[ct][:, 2 * C:3 * C],
                                      wqkv_d[ct * P:(ct + 1) * P, 2 * C:3 * C])
                for hp in range(H // 2):
                    for lo in (hp * P, C + hp * P):   # q col, then k col
                        for ct in range(CT):
                            nc.sync.dma_start(
                                w_tiles[ct][:, lo:lo + P],
                                wqkv_d[ct * P:(ct + 1) * P, lo:lo + P])
                for ct in range(CT):
                    nc.sync.dma_start(wp_tiles[ct][:],
                                      wproj_d[ct * P:(ct + 1) * P, :])

                for t in range(TT):
                    # ones column at index h*65+64 for each head
                    nc.vector.tensor_copy(
                        v_all[t].rearrange("p (h c) -> p h c", c=D + 1)[:, :, D],
                        ones_f[:])

                # v token-major for all token tiles
                for tt in range(TT):
                    lo = tt * P
                    psv = pp_v.tile([P, C], F32, tag="v", name=f"vps{tt}")
                    for off, wd in ((0, 512), (512, 256)):
                        vsl = slice(2 * C + off, 2 * C + off + wd)
                        for i, ct in enumerate(range(CT)):
                            nc.tensor.matmul(
                                psv[:, off:off + wd],
                                xT[ct][:, lo:lo + P],
                                w_tiles[ct][:, vsl],
                                start=(i == 0), stop=(i == CT - 1))
                    # one strided copy: psv [p, (h d)] -> v_all [p, (h 65)]
                    nc.vector.tensor_copy(
                        v_all[tt].rearrange("p (h c) -> p h c", c=D + 1)[:, :, 0:D],
                        psv.rearrange("p (h d) -> p h d", d=D))

                # q,k feature-major, PAIR-MAJOR so attention can start early
                for hp in range(H // 2):
                    for cp in (hp, 6 + hp):
                        for t2 in range(T2):
                            ts2 = slice(t2 * 512, (t2 + 1) * 512)
                            ps = pp_qk.tile([P, 512], F32, tag="qk",
                                            name=f"qk{cp}_{t2}")
                            for i, ct in enumerate(range(CT)):
                                nc.tensor.matmul(
                                    ps[:],
                                    w_tiles[ct][:, cp * P:(cp + 1) * P],
                                    xT[ct][:, ts2],
                                    start=(i == 0), stop=(i == CT - 1))
                            nc.vector.tensor_copy(qkT[cp][:, ts2], ps[:])

        # ---------------- Phase 2: attention ----------------
        with tc.tile_pool(name="ph2", bufs=1) as ph2, \
             tc.tile_pool(name="pp_sc", bufs=2, space="PSUM") as pp_sc, \
             tc.tile_pool(name="pp_po", bufs=4, space="PSUM") as pp_po:
            for _rep in range(repeat if 2 in phases else 0):

                for hp in range(H // 2):
                    qt, kt = qkT[hp], qkT[6 + hp]
                    po = [[pp_po.tile([D + 1, 512], F32, tag="po",
                                      name=f"po{hp}_{hh}_{i2}")
                           for i2 in range(T2)] for hh in range(2)]
                    # per-head score PSUM tiles, reused across jt within the
                    # pair: scores(jt+1) of head X WAR-wait only on exp of
                    # head X at jt, and exp_A overlaps head B's score MMs
                    sc_h = [pp_sc.tile([P, N], F32, tag=f"sc{hh}", bufs=1,
                                       name=f"sc{hp}_{hh}")
                            for hh in range(2)]
                    # software-pipelined: scores/exp for jt run one step
                    # ahead of the PV matmuls of jt-1
                    prs = {}

                    def emit_scores_exp(jt):
                        jsl = slice(jt * P, (jt + 1) * P)
                        # row-tile paired score matmuls: head A rows 0:64,
                        # head B rows 64:128 run concurrently on the PE;
                        # issue order A0,B0,A1,B1 keeps both row groups hot
                        for i2 in range(T2):
                            for hh in range(2):
                                lo = hh * 64
                                nc.tensor.matmul(
                                    sc_h[hh][:, i2 * 512:(i2 + 1) * 512],
                                    kt[lo:lo + 64, jsl],
                                    qt[lo:lo + 64, i2 * 512:(i2 + 1) * 512],
                                    start=True, stop=True)
                        pra = ph2.tile([P, N], BF16, tag="pra", bufs=3,
                                       name=f"pra{hp}_{jt}")
                        prb = ph2.tile([P, N], BF16, tag="prb", bufs=3,
                                       name=f"prb{hp}_{jt}")
                        nc.scalar.activation(
                            pra[:], sc_h[0][:],
                            mybir.ActivationFunctionType.Exp,
                            scale=float(D) ** -0.5)
                        nc.scalar.activation(
                            prb[:], sc_h[1][:],
                            mybir.ActivationFunctionType.Exp,
                            scale=float(D) ** -0.5)
                        prs[jt] = (pra, prb)

                    def emit_pv(jt):
                        if ph2_parts == "se":
                            return
                        parts = prs.pop(jt)
                        for hh in range(2):
                            h = 2 * hp + hh
                            va = v_all[jt][:, h * (D + 1):(h + 1) * (D + 1)]
                            for i2 in range(T2):
                                nc.tensor.matmul(
                                    po[hh][i2][:], va,
                                    parts[hh][:, i2 * 512:(i2 + 1) * 512],
                                    start=(jt == 0), stop=(jt == TT - 1))

                    emit_scores_exp(0)
                    for jt in range(1, TT):
                        emit_scores_exp(jt)
                        emit_pv(jt - 1)
                    emit_pv(TT - 1)
                    if ph2_parts == "se":
                        for hh in range(2):
                            for i2 in range(T2):
                                nc.tensor.matmul(po[hh][i2][:],
                                                 v_all[0][:, 0:D + 1],
                                                 qkT[6][:, 0:512],
                                                 start=True, stop=True)
                    if ph2_parts in ("se", "sepv"):
                        # minimal eviction so PSUM frees; skip normalization
                        for hh in range(2):
                            for i2 in range(T2):
                                o = ph2.tile([D, 512], BF16, tag="ot",
                                             bufs=4, name=f"ot{hp}_{hh}_{i2}")
                                nc.vector.tensor_copy(o[:], po[hh][i2][0:D, :])
                        continue

                    # denominator rows straight from PSUM to pk (no wait on
                    # the eviction), evict PV accumulators as bf16, then
                    # normalize.  1/den computed on ACT as exp(-ln(den)),
                    # batched [4, 512] per pair (Ln/Exp share the loaded act
                    # table with the softmax Exp, so no table reloads).
                    ot = [[None] * T2, [None] * T2]
                    pk = ph2.tile([4, 512], BF16, tag="pk", bufs=2,
                                  name=f"pk{hp}")
                    rk = ph2.tile([4, 512], F32, tag="rk", bufs=2,
                                  name=f"rk{hp}")
                    rkb = ph2.tile([4, 512], BF16, tag="rkb", bufs=2,
                                   name=f"rkb{hp}")
                    for hh in range(2):
                        for i2 in range(T2):
                            o = ph2.tile([D + 1, 512], BF16, tag="ot", bufs=4,
                                         name=f"ot{hp}_{hh}_{i2}")
                            nc.vector.tensor_copy(o[:], po[hh][i2][:])
                            ot[hh][i2] = o
                            # hop den row to partition (2*hh + i2) of pk
                            nc.sync.dma_start(
                                pk[2 * hh + i2:2 * hh + i2 + 1, :],
                                o[64:65, :])
                    nc.scalar.activation(rk[:], pk[:],
                                         mybir.ActivationFunctionType.Ln)
                    nc.scalar.activation(rk[:], rk[:],
                                         mybir.ActivationFunctionType.Exp,
                                         scale=-1.0)
                    # recip is only bf16-rounded AFTER the exp(-ln(den))
                    # (rounding ln(den) itself would amplify to ~3% error)
                    nc.vector.tensor_copy(rkb[:], rk[:])
                    for hh in range(2):
                        for i2 in range(T2):
                            isl = slice(i2 * 512, (i2 + 1) * 512)
                            o = ot[hh][i2]
                            r = 2 * hh + i2
                            rc0 = ph2.tile([1, 512], BF16, tag="rc0",
                                           bufs=4, name=f"rc0{hp}_{hh}_{i2}")
                            nc.sync.dma_start(rc0[:], rkb[r:r + 1, :])
                            bcs = ph2.tile([64, 512], BF16, tag="bcs", bufs=2,
                                           name=f"bcs{hp}_{hh}_{i2}")
                            nc.gpsimd.partition_broadcast(bcs[:], rc0[:])
                            if hh == 0:
                                nc.vector.tensor_mul(attnT[hp][0:64, isl],
                                                     o[0:64, :], bcs[:])
                            else:
                                nt = ph2.tile([64, 512], BF16, tag="nt",
                                              bufs=2, name=f"nt{hp}_{i2}")
                                nc.vector.tensor_mul(nt[:], o[0:64, :], bcs[:])
                                nc.sync.dma_start(attnT[hp][64:128, isl],
                                                  nt[:])

        # ------------- Phase 3: output projection (LoRA pre-folded) -------------
        with tc.tile_pool(name="ph3", bufs=1) as ph3, \
             tc.tile_pool(name="pp_y", bufs=3, space="PSUM") as pp_y:
            for _rep in range(repeat if 3 in phases else 0):
                for t2 in range(T2):
                    ts2 = slice(t2 * 512, (t2 + 1) * 512)
                    for cp in range(CT):
                        ps = pp_y.tile([P, 512], F32, tag="y", name=f"y{t2}_{cp}")
                        # contract over early head-pairs first; attnT[4]/[5]
                        # finish last in phase 2, so keep them at chain end
                        order = [((cp + 2 * t2) % 4 + k) % 4
                                 for k in range(4)] + [4, 5]
                        for i, ct in enumerate(order):
                            nc.tensor.matmul(
                                ps[:], wp_tiles[ct][:, cp * P:(cp + 1) * P],
                                attnT[ct][:, ts2],
                                start=(i == 0), stop=(i == CT - 1))
                        st = ph3.tile([P, 512], BF16, tag="st", bufs=4,
                                      name=f"st{t2}_{cp}")
                        # bias add on DVE (per-partition scalar operand)
                        nc.vector.tensor_scalar(
                            st[:], ps[:], bvec[:, cp:cp + 1], None,
                            mybir.AluOpType.add)
                        nc.sync.dma_start(out_d[cp * P:(cp + 1) * P, ts2],
                                          st[:])
    nc.compile()
    return nc


_NC = None
_JITTED = None
_META = None


def _get_nc():
    global _NC
    if _NC is None:
        _NC = build_nc()
    return _NC


def _build_runner():
    global _JITTED, _META
    if _JITTED is not None:
        return
    from jax.experimental.shard_map import shard_map
    from jax.sharding import Mesh, PartitionSpec
    from concourse.bass2jax import (install_neuronx_cc_hook, _bass_exec_p,
                                    partition_id_tensor)

    nc = _get_nc()
    install_neuronx_cc_hook()

    partition_name = (nc.partition_id_tensor.name
                      if nc.partition_id_tensor else None)
    in_names, out_names, out_avals, zero_outs = [], [], [], []
    for alloc in nc.m.functions[0].allocations:
        if not isinstance(alloc, mybir.MemoryLocationSet):
            continue
        name = alloc.memorylocations[0].name
        if alloc.kind == "ExternalInput":
            if name == partition_name:
                continue
            in_names.append(name)
        elif alloc.kind == "ExternalOutput":
            out_names.append(name)
            shape = tuple(alloc.tensor_shape)
            dtype = mybir.dt.np(alloc.dtype)
            out_avals.append(jax.core.ShapedArray(shape, dtype))
            zero_outs.append(np.zeros(shape, dtype))
    n_params = len(in_names)
    all_names = in_names + out_names
    if partition_name is not None:
        all_names = all_names + [partition_name]
    donate = tuple(range(n_params, n_params + len(out_names)))

    def _body(*args):
        operands = list(args)
        if partition_name is not None:
            operands.append(partition_id_tensor())
        outs = _bass_exec_p.bind(
            *operands,
            out_avals=tuple(out_avals),
            in_names=tuple(all_names),
            out_names=tuple(out_names),
            lowering_input_output_aliases=(),
            sim_require_finite=True,
            sim_require_nnan=True,
            nc=nc,
        )
        return tuple(outs)

    devices = jax.devices()[:N_CORES]
    mesh = Mesh(np.asarray(devices), ("core",))
    specs = (PartitionSpec("core"),) * (n_params + len(out_names))
    _JITTED = jax.jit(
        shard_map(_body, mesh=mesh, in_specs=specs,
                  out_specs=(PartitionSpec("core"),) * len(out_names),
                  check_rep=False),
        donate_argnums=donate, keep_unused=True)
    _META = (in_names, out_names, zero_outs)


def make_in_maps(x, W_qkv, W_proj, b_proj, A_qkv, B_qkv, A_proj, B_proj):
    x = np.asarray(x, dtype=np.float32)
    W_qkv = np.asarray(W_qkv, dtype=np.float32)
    W_proj = np.asarray(W_proj, dtype=np.float32)
    b_proj = np.asarray(b_proj, dtype=np.float32)
    A_qkv = np.asarray(A_qkv, dtype=np.float32)
    B_qkv = np.asarray(B_qkv, dtype=np.float32)
    A_proj = np.asarray(A_proj, dtype=np.float32)
    B_proj = np.asarray(B_proj, dtype=np.float32)

    # fold LoRA into the weights (exact algebra, fp32 on host)
    wqkv_eff = W_qkv + LORA_SCALING * (A_qkv @ B_qkv)
    wproj_eff = W_proj + LORA_SCALING * ((W_proj @ A_proj) @ B_proj)
    b_eff = b_proj + LORA_SCALING * (B_proj.T @ (A_proj.T @ b_proj))

    bf = mybir.dt.np(BF16)
    reps = {
        "W_qkv": np.ascontiguousarray(wqkv_eff).astype(bf),
        "W_proj": np.ascontiguousarray(wproj_eff).astype(bf),
        "b_proj": np.ascontiguousarray(b_eff),
    }
    return [
        {"x": np.ascontiguousarray(x[b].T).astype(bf), **reps}
        for b in range(N_CORES)
    ]


def kernel(x, W_qkv, W_proj, b_proj, A_qkv, B_qkv, A_proj, B_proj):
    _build_runner()
    in_names, out_names, zero_outs = _META
    in_maps = make_in_maps(x, W_qkv, W_proj, b_proj, A_qkv, B_qkv,
                           A_proj, B_proj)
    per_core = [[np.asarray(m[name]) for name in in_names] for m in in_maps]
    concat_in = [
        np.concatenate([per_core[c][i] for c in range(N_CORES)], axis=0)
        for i in range(len(in_names))
    ]
    concat_zero = [
        np.concatenate([z] * N_CORES, axis=0) for z in zero_outs
    ]
    out_arrs = _JITTED(*concat_in, *concat_zero)
    out = np.asarray(out_arrs[0]).astype(np.float32)  # [8*768, 1024]
    return np.ascontiguousarray(
        out.reshape(B, C, N).transpose(0, 2, 1)).astype(np.float32)
